# revision 13
# baseline (speedup 1.0000x reference)
"""Trainium2 Bass kernel for the scatter_memory GRU memory-update module.

Computation (torch GRUCell semantics, chunk order r, z, n):
    current = memory[node_ids]                       # [B, H] gather
    gi = messages @ W_ih.T + b_ih ; gh = current @ W_hh.T + b_hh
    r = sigmoid(gi_r + gh_r) ; z = sigmoid(gi_z + gh_z)
    n = tanh(gi_n + r * gh_n)
    updated = (1 - z) * n + z * current
    new_memory = memory.at[node_ids].set(updated)    # scatter

Distribution: the B updated rows are sharded contiguously across 8
NeuronCores.  The gather/scatter over the 500k-row table and the
feature-major transposes run on the host; each core runs the GRU math on
its own [H, B/8] shard (feature dim H=128 sits on the SBUF partition
axis, so the GRU biases become per-partition vectors that fuse into the
ScalarEngine activation ops for free).

Engine-balance restructure (v2):
  * w = sigmoid(-a_z) = 1 - z computed directly by the ACT op with
    scale=-1 and a negated bias column, so the output blend is
        out = h + w * (n - h)
    (one DVE sub, one DVE mul, one GpSimd add - no z*h GpSimd multiply
    and no (z-1)*n STT).
  * The n-gate preactivation is accumulated IN PSUM: the W_hh_n matmul
    writes p_hn, one in-place DVE STT rewrites it to (p_hn + b_hn)*r,
    then the W_ih_n matmul accumulates i_n on top (start=False).  This
    removes the separate `pre = t + i_n` DVE add.
  * PE / ACT instruction streams are software-pipelined one group deep
    so the late i_n matmul / tanh never stall the in-order engines.
"""

import os
import sys

import numpy as np

for _p in ("/opt/trn_rl_repo", "/root/.axon_site/_ro/trn_rl_repo"):
    if os.path.isdir(_p) and _p not in sys.path:
        sys.path.insert(0, _p)

import ml_dtypes
from contextlib import ExitStack

import concourse.bass as bass
import concourse.tile as tile
from concourse import mybir
from concourse.bass_utils import run_bass_kernel_spmd

BF16 = ml_dtypes.bfloat16
import json as _json

N_CORES = 8
H = 128
NTILE = 1024         # batch columns per pipeline group (2 PSUM banks/gate)
CHUNK = 2048         # batch columns per DMA chunk

# exposed for test harnesses
LAST_RESULT = None

_NC_CACHE = {}


def _split_sync_waits(bir: dict) -> dict:
    """Hoist extra per-instruction semaphore waits into standalone
    EventSemaphore instructions.

    The walrus build in this container encodes at most ONE sync wait per
    instruction ("Too many sync wait commands" otherwise); Tile attaches
    one wait per dependency.  An engine-level standalone wait immediately
    before the instruction is semantically identical (the engine stalls
    either way), so keep the last wait inline and hoist the rest.
    """
    n = 0
    for fn in bir.get("functions", []):
        for blk in fn.get("blocks", []):
            out = []
            for inst in blk.get("instructions", []):
                si = inst.get("sync_info") or {}
                ow = si.get("on_wait") or []
                if len(ow) > 1:
                    for w in ow[:-1]:
                        n += 1
                        out.append({
                            "debug": inst.get("debug", 0),
                            "engine": inst["engine"],
                            "ins": [],
                            "outs": [],
                            "name": f"hoistw_{n}_{inst['name']}",
                            "opcode": "EventSemaphore",
                            "sync_info": {"on_update": [], "on_wait": [w]},
                        })
                    si["on_wait"] = [ow[-1]]
                out.append(inst)
            blk["instructions"] = out
    return bir


def _patch_json(nc: bass.Bass) -> None:
    orig = nc.to_json_bytes

    def patched() -> bytes:
        return _json.dumps(_split_sync_waits(_json.loads(orig()))).encode()

    nc.to_json_bytes = patched


def _build_nc(bpc: int) -> bass.Bass:
    """Bass program for one core: GRU over a [H, bpc] feature-major shard."""
    assert bpc % 512 == 0 and CHUNK % NTILE == 0
    f32 = mybir.dt.float32
    bf16 = mybir.dt.bfloat16
    sig = mybir.ActivationFunctionType.Sigmoid
    tanh = mybir.ActivationFunctionType.Tanh
    add_op = mybir.AluOpType.add
    mult_op = mybir.AluOpType.mult

    nc = bass.Bass()
    xT = nc.declare_dram_parameter("xT", [H, bpc], bf16, isOutput=False)
    hT = nc.declare_dram_parameter("hT", [H, bpc], bf16, isOutput=False)
    w_ihT = nc.declare_dram_parameter("w_ihT", [H, 3 * H], bf16, isOutput=False)
    w_hhT = nc.declare_dram_parameter("w_hhT", [H, 3 * H], bf16, isOutput=False)
    # bias columns: 0 = b_ih_r + b_hh_r, 1 = -(b_ih_z + b_hh_z),
    #               2 = b_hh_n, 3 = b_ih_n
    biases = nc.declare_dram_parameter("biases", [H, 4], f32, isOutput=False)
    outT = nc.declare_dram_parameter("outT", [H, bpc], bf16, isOutput=True)

    # groups of NTILE columns; small first group so compute starts while
    # the big DMAs stream, small last groups so the serial tail is short
    group_bounds = []
    pos = 0
    first = min(512, bpc)
    group_bounds.append((0, first))
    pos = first
    while bpc - pos > NTILE:
        if bpc - pos <= NTILE + 512:  # leave >=512 for the final group
            break
        group_bounds.append((pos, NTILE))
        pos += NTILE
    while pos < bpc:
        gsz = min(512, bpc - pos)
        group_bounds.append((pos, gsz))
        pos += gsz
    assert pos == bpc
    n_groups = len(group_bounds)

    with ExitStack() as ctx:
        tc = ctx.enter_context(tile.TileContext(nc))
        singles = ctx.enter_context(tc.tile_pool(name="singles", bufs=1))
        io = ctx.enter_context(tc.tile_pool(name="io", bufs=8))
        outp = ctx.enter_context(tc.tile_pool(name="outp", bufs=3))
        mids = ctx.enter_context(tc.tile_pool(name="mids", bufs=3))
        # PSUM: p_r 2 banks + p_z 2 banks + p_hn 2x2 banks = all 8 banks
        psum = ctx.enter_context(tc.tile_pool(name="psum", bufs=1, space="PSUM"))
        psum_hn = ctx.enter_context(
            tc.tile_pool(name="psum_hn", bufs=2, space="PSUM"))

        # small one-time loads go through SWDGE (single queue) so consumers
        # don't accumulate one sem wait per HWDGE hardware queue
        w_ih_sb = singles.tile([H, 3 * H], bf16)
        nc.gpsimd.dma_start(out=w_ih_sb, in_=w_ihT[:, :])
        w_hh_sb = singles.tile([H, 3 * H], bf16)
        nc.gpsimd.dma_start(out=w_hh_sb, in_=w_hhT[:, :])
        b_sb = singles.tile([H, 4], f32)
        nc.gpsimd.dma_start(out=b_sb, in_=biases[:, :])

        # dummy sigmoid fires the ~2.7us ACT table load immediately, so it
        # overlaps the DMA ramp instead of stalling the first real sigmoid
        warm_sb = singles.tile([H, 1], f32)
        nc.scalar.activation(out=warm_sb, in_=b_sb[:, 0:1],
                             func=sig, bias=0.0, scale=1.0)

        # per-group state carried across the software pipeline
        groups = [dict() for _ in range(n_groups)]

        # DMA chunks: all of x/h streamed in CHUNK-col pieces up front via
        # the HW queues; tile deps gate each group's matmuls on its chunk
        # pack whole groups into DMA chunks of <= CHUNK columns
        chunk_groups = []
        cur = []
        cur_sz = 0
        for gi, (g0, gsz) in enumerate(group_bounds):
            if cur and cur_sz + gsz > CHUNK:
                chunk_groups.append(cur)
                cur, cur_sz = [], 0
            cur.append(gi)
            cur_sz += gsz
        if cur:
            chunk_groups.append(cur)

        x_chunks = {}
        h_chunks = {}
        o_chunks = {}
        chunk_of = {}
        for ci, gis in enumerate(chunk_groups):
            pos = group_bounds[gis[0]][0]
            csz = sum(group_bounds[g][1] for g in gis)
            x_sb = io.tile([H, csz], bf16, tag="x")
            h_sb = io.tile([H, csz], bf16, tag="h")
            nc.sync.dma_start(out=x_sb, in_=xT[:, pos : pos + csz])
            nc.sync.dma_start(out=h_sb, in_=hT[:, pos : pos + csz])
            o_sb = outp.tile([H, csz], bf16, tag="o")
            for gi in gis:
                chunk_of[gi] = (ci, pos, csz)
                x_chunks[gi] = x_sb
                h_chunks[gi] = h_sb
                o_chunks[gi] = o_sb
        n_chunks = len(chunk_groups)

        W_R, W_Z, W_N = slice(0, H), slice(H, 2 * H), slice(2 * H, 3 * H)

        def emit_gates_mm(g):
            """r/z/hn gate matmuls for group g (10 MMs of FD=512)."""
            st = groups[g]
            x_sb, h_sb = x_chunks[g], h_chunks[g]
            _, cpos, _ = chunk_of[g]
            gstart, gsz = group_bounds[g]
            off = gstart - cpos
            p_r = psum.tile([H, NTILE], f32, tag="p_r")
            p_z = psum.tile([H, NTILE], f32, tag="p_z")
            p_hn = psum_hn.tile([H, NTILE], f32, tag="p_hn")
            st.update(p_r=p_r, p_z=p_z, p_hn=p_hn, off=off, gsz=gsz)
            # all r-gate matmuls first so the sigmoid can start while the
            # z / h_n matmuls are still running
            for q0 in range(0, gsz, 512):
                qs = slice(off + q0, off + q0 + 512)
                qd = slice(q0, q0 + 512)
                nc.tensor.matmul(p_r[:, qd], w_ih_sb[:, W_R], x_sb[:, qs],
                                 start=True, stop=False)
                nc.tensor.matmul(p_r[:, qd], w_hh_sb[:, W_R], h_sb[:, qs],
                                 start=False, stop=True)
            for q0 in range(0, gsz, 512):
                qs = slice(off + q0, off + q0 + 512)
                qd = slice(q0, q0 + 512)
                nc.tensor.matmul(p_z[:, qd], w_ih_sb[:, W_Z], x_sb[:, qs],
                                 start=True, stop=False)
                nc.tensor.matmul(p_z[:, qd], w_hh_sb[:, W_Z], h_sb[:, qs],
                                 start=False, stop=True)
            for q0 in range(0, gsz, 512):
                qs = slice(off + q0, off + q0 + 512)
                qd = slice(q0, q0 + 512)
                nc.tensor.matmul(p_hn[:, qd], w_hh_sb[:, W_N], h_sb[:, qs],
                                 start=True, stop=True)

        def emit_in_mm(g):
            """i_n matmuls accumulating onto (h_n + b)*r already in PSUM."""
            st = groups[g]
            x_sb = x_chunks[g]
            off = st["off"]
            p_hn = st["p_hn"]
            for q0 in range(0, st["gsz"], 512):
                nc.tensor.matmul(p_hn[:, q0 : q0 + 512],
                                 w_ih_sb[:, W_N],
                                 x_sb[:, off + q0 : off + q0 + 512],
                                 start=False, stop=True)

        def emit_sig_r(g):
            st = groups[g]
            gsz = st["gsz"]
            r_t = mids.tile([H, NTILE], bf16, tag="r")
            nc.scalar.activation(out=r_t[:, :gsz], in_=st["p_r"][:, :gsz],
                                 func=sig, bias=b_sb[:, 0:1], scale=1.0)
            st["r"] = r_t

        def emit_sig_w(g):
            st = groups[g]
            gsz = st["gsz"]
            w_t = mids.tile([H, NTILE], bf16, tag="w")
            nc.scalar.activation(out=w_t[:, :gsz], in_=st["p_z"][:, :gsz],
                                 func=sig, bias=b_sb[:, 1:2], scale=-1.0)
            st["w"] = w_t

        def emit_stt(g):
            """In-place PSUM rewrite: p_hn <- (p_hn + b_hn) * r."""
            st = groups[g]
            gsz = st["gsz"]
            nc.vector.scalar_tensor_tensor(
                out=st["p_hn"][:, :gsz], in0=st["p_hn"][:, :gsz],
                scalar=b_sb[:, 2:3], in1=st["r"][:, :gsz],
                op0=add_op, op1=mult_op)

        def emit_tanh(g):
            st = groups[g]
            gsz = st["gsz"]
            n_t = mids.tile([H, NTILE], bf16, tag="n")
            nc.scalar.activation(out=n_t[:, :gsz], in_=st["p_hn"][:, :gsz],
                                 func=tanh, bias=b_sb[:, 3:4], scale=1.0)
            st["n"] = n_t

        def emit_blend(g):
            """out = h + w * (n - h): DVE sub, DVE mul, GpSimd add."""
            st = groups[g]
            h_sb, o_sb = h_chunks[g], o_chunks[g]
            off, gsz = st["off"], st["gsz"]
            hs = slice(off, off + gsz)
            m_t = mids.tile([H, NTILE], bf16, tag="m")
            d_t = mids.tile([H, NTILE], bf16, tag="d")
            nc.vector.tensor_sub(out=m_t[:, :gsz], in0=st["n"][:, :gsz],
                                 in1=h_sb[:, hs])
            nc.vector.tensor_mul(out=d_t[:, :gsz], in0=st["w"][:, :gsz],
                                 in1=m_t[:, :gsz])
            nc.gpsimd.tensor_add(out=o_sb[:, hs], in0=d_t[:, :gsz],
                                 in1=h_sb[:, hs])
            # drop references so tile pool can recycle
            st.clear()

        def emit_out_dma(ci_, cpos, csz, o_sb):
            nc.sync.dma_start(out=outT[:, cpos : cpos + csz], in_=o_sb)

        # software pipeline, one group deep on the PE/ACT streams:
        #   g:   gates MM(g), sigmoids(g), stt(g)
        #        [then next iteration emits gates MM(g+1) before in_mm(g)]
        #   g-1 style: in_mm(g), tanh(g), blend(g)
        last_group_of_chunk = {}
        for gi in range(n_groups):
            last_group_of_chunk[chunk_of[gi][0]] = gi

        def maybe_out_dma(g):
            ci_, cpos, csz = chunk_of[g]
            if last_group_of_chunk[ci_] == g:
                emit_out_dma(ci_, cpos, csz, o_chunks[g])

        # Steady-state iteration g (one group deep on PE/ACT/DVE streams):
        #   PE:  in_mm(g-1) first (tanh(g-1) needs it early), then gates(g)
        #   ACT: sig_r(g), tanh(g-1), sig_w(g)  - tanh's STT/in_mm chain had
        #        a full iteration of slack, so ACT runs back-to-back
        #   DVE: stt(g) (after sig_r), then blend muls of g-1 (after tanh)
        #   GpSimd: final add of g-1
        for g in range(n_groups):
            if g > 0:
                emit_in_mm(g - 1)
            emit_gates_mm(g)
            emit_sig_r(g)
            if g > 0:
                emit_tanh(g - 1)
            emit_sig_w(g)
            emit_stt(g)
            if g > 0:
                emit_blend(g - 1)
                maybe_out_dma(g - 1)
        g = n_groups - 1
        emit_in_mm(g)
        emit_tanh(g)
        emit_blend(g)
        maybe_out_dma(g)

    _patch_json(nc)
    return nc


def _get_nc(bpc: int) -> bass.Bass:
    if bpc not in _NC_CACHE:
        _NC_CACHE[bpc] = _build_nc(bpc)
    return _NC_CACHE[bpc]


def kernel(node_ids, messages, memory, W_ih, W_hh, b_ih, b_hh):
    global LAST_RESULT
    node_ids = np.asarray(node_ids)
    messages = np.asarray(messages, dtype=np.float32)
    memory = np.asarray(memory, dtype=np.float32)
    W_ih = np.asarray(W_ih, dtype=np.float32)
    W_hh = np.asarray(W_hh, dtype=np.float32)
    b_ih = np.asarray(b_ih, dtype=np.float32)
    b_hh = np.asarray(b_hh, dtype=np.float32)

    B = node_ids.shape[0]
    per = -(-B // N_CORES)                       # rows per core (unpadded)
    bpc = -(-per // 512) * 512                   # padded to 512 multiple
    nc = _get_nc(bpc)

    current = memory[node_ids]                   # [B, H] host gather

    w_ihT = np.ascontiguousarray(W_ih.T).astype(BF16)
    w_hhT = np.ascontiguousarray(W_hh.T).astype(BF16)
    bias = np.empty((H, 4), dtype=np.float32)
    bias[:, 0] = b_ih[0:H] + b_hh[0:H]
    bias[:, 1] = -(b_ih[H : 2 * H] + b_hh[H : 2 * H])
    bias[:, 2] = b_hh[2 * H : 3 * H]
    bias[:, 3] = b_ih[2 * H : 3 * H]

    in_maps = []
    for c in range(N_CORES):
        lo = c * per
        hi = min(lo + per, B)
        xT = np.zeros((H, bpc), dtype=BF16)
        hT = np.zeros((H, bpc), dtype=BF16)
        if hi > lo:
            xT[:, : hi - lo] = messages[lo:hi].T
            hT[:, : hi - lo] = current[lo:hi].T
        in_maps.append({
            "xT": xT, "hT": hT,
            "w_ihT": w_ihT, "w_hhT": w_hhT, "biases": bias,
        })

    res = run_bass_kernel_spmd(nc, in_maps, list(range(N_CORES)))
    LAST_RESULT = res

    updated = np.empty((B, H), dtype=np.float32)
    for c in range(N_CORES):
        lo = c * per
        hi = min(lo + per, B)
        if hi > lo:
            updated[lo:hi] = res.results[c]["outT"][:, : hi - lo].T.astype(np.float32)

    new_memory = memory.copy()
    new_memory[node_ids] = updated
    return new_memory


# revision 20
# speedup vs baseline: 1.0451x; 1.0451x over previous
"""Trainium2 Bass kernel for the scatter_memory GRU memory-update module.

Computation (torch GRUCell semantics, chunk order r, z, n):
    current = memory[node_ids]                       # [B, H] gather
    gi = messages @ W_ih.T + b_ih ; gh = current @ W_hh.T + b_hh
    r = sigmoid(gi_r + gh_r) ; z = sigmoid(gi_z + gh_z)
    n = tanh(gi_n + r * gh_n)
    updated = (1 - z) * n + z * current
    new_memory = memory.at[node_ids].set(updated)    # scatter

Distribution: the B updated rows are sharded contiguously across 8
NeuronCores.  The gather/scatter over the 500k-row table and the
feature-major transposes run on the host; each core runs the GRU math on
its own [H, B/8] shard (feature dim H=128 sits on the SBUF partition
axis, so the GRU biases become per-partition vectors that fuse into the
ScalarEngine activation ops for free).

Engine-balance restructure (v2):
  * w = sigmoid(-a_z) = 1 - z computed directly by the ACT op with
    scale=-1 and a negated bias column, so the output blend is
        out = h + w * (n - h)
    (one DVE sub, one DVE mul, one GpSimd add - no z*h GpSimd multiply
    and no (z-1)*n STT).
  * The n-gate preactivation is accumulated IN PSUM: the W_hh_n matmul
    writes p_hn, one in-place DVE STT rewrites it to (p_hn + b_hn)*r,
    then the W_ih_n matmul accumulates i_n on top (start=False).  This
    removes the separate `pre = t + i_n` DVE add.
  * PE / ACT instruction streams are software-pipelined one group deep
    so the late i_n matmul / tanh never stall the in-order engines.
"""

import os
import sys

import numpy as np

for _p in ("/opt/trn_rl_repo", "/root/.axon_site/_ro/trn_rl_repo"):
    if os.path.isdir(_p) and _p not in sys.path:
        sys.path.insert(0, _p)

import ml_dtypes
from contextlib import ExitStack

import concourse.bass as bass
import concourse.tile as tile
from concourse import mybir
from concourse.bass_utils import run_bass_kernel_spmd

BF16 = ml_dtypes.bfloat16
import json as _json

N_CORES = 8
H = 128
NTILE = 1024         # batch columns per pipeline group (2 PSUM banks/gate)
CHUNK = 2048         # batch columns per DMA chunk

# exposed for test harnesses
LAST_RESULT = None

_NC_CACHE = {}


def _split_sync_waits(bir: dict) -> dict:
    """Hoist extra per-instruction semaphore waits into standalone
    EventSemaphore instructions.

    The walrus build in this container encodes at most ONE sync wait per
    instruction ("Too many sync wait commands" otherwise); Tile attaches
    one wait per dependency.  An engine-level standalone wait immediately
    before the instruction is semantically identical (the engine stalls
    either way), so keep the last wait inline and hoist the rest.
    """
    n = 0
    for fn in bir.get("functions", []):
        for blk in fn.get("blocks", []):
            out = []
            for inst in blk.get("instructions", []):
                si = inst.get("sync_info") or {}
                ow = si.get("on_wait") or []
                if len(ow) > 1:
                    for w in ow[:-1]:
                        n += 1
                        out.append({
                            "debug": inst.get("debug", 0),
                            "engine": inst["engine"],
                            "ins": [],
                            "outs": [],
                            "name": f"hoistw_{n}_{inst['name']}",
                            "opcode": "EventSemaphore",
                            "sync_info": {"on_update": [], "on_wait": [w]},
                        })
                    si["on_wait"] = [ow[-1]]
                out.append(inst)
            blk["instructions"] = out
    return bir


def _patch_json(nc: bass.Bass) -> None:
    orig = nc.to_json_bytes

    def patched() -> bytes:
        return _json.dumps(_split_sync_waits(_json.loads(orig()))).encode()

    nc.to_json_bytes = patched


def _build_nc(bpc: int, used: int) -> bass.Bass:
    """Bass program for one core: GRU over a [H, bpc] feature-major shard.

    Buffers are sized/padded to `bpc` columns; only the first `used`
    columns are actually computed and written back.
    """
    assert bpc % 512 == 0 and CHUNK % NTILE == 0 and 0 < used <= bpc
    f32 = mybir.dt.float32
    bf16 = mybir.dt.bfloat16
    sig = mybir.ActivationFunctionType.Sigmoid
    tanh = mybir.ActivationFunctionType.Tanh
    add_op = mybir.AluOpType.add
    mult_op = mybir.AluOpType.mult

    nc = bass.Bass()
    xT = nc.declare_dram_parameter("xT", [H, bpc], bf16, isOutput=False)
    hT = nc.declare_dram_parameter("hT", [H, bpc], bf16, isOutput=False)
    w_ihT = nc.declare_dram_parameter("w_ihT", [H, 3 * H], bf16, isOutput=False)
    w_hhT = nc.declare_dram_parameter("w_hhT", [H, 3 * H], bf16, isOutput=False)
    # bias columns: 0 = b_ih_r + b_hh_r, 1 = -(b_ih_z + b_hh_z),
    #               2 = b_hh_n, 3 = b_ih_n
    biases = nc.declare_dram_parameter("biases", [H, 4], f32, isOutput=False)
    outT = nc.declare_dram_parameter("outT", [H, bpc], bf16, isOutput=True)

    # groups of NTILE columns; small leading groups so compute starts while
    # the big DMAs stream, small trailing groups so the serial tail is short
    group_bounds = []
    pos = 0
    for lead in (256, 512):
        if pos < used and used - pos > NTILE:
            gsz = min(lead, used - pos)
            group_bounds.append((pos, gsz))
            pos += gsz
    while used - pos > NTILE + 512:
        group_bounds.append((pos, NTILE))
        pos += NTILE
    while pos < used:
        gsz = min(512, used - pos)
        group_bounds.append((pos, gsz))
        pos += gsz
    assert pos == used
    n_groups = len(group_bounds)

    with ExitStack() as ctx:
        tc = ctx.enter_context(tile.TileContext(nc))
        singles = ctx.enter_context(tc.tile_pool(name="singles", bufs=1))
        io = ctx.enter_context(tc.tile_pool(name="io", bufs=8))
        outp = ctx.enter_context(tc.tile_pool(name="outp", bufs=3))
        mids = ctx.enter_context(tc.tile_pool(name="mids", bufs=3))
        # PSUM: p_r 2 banks + p_z 2 banks + p_hn 2x2 banks = all 8 banks
        psum = ctx.enter_context(tc.tile_pool(name="psum", bufs=1, space="PSUM"))
        psum_hn = ctx.enter_context(
            tc.tile_pool(name="psum_hn", bufs=2, space="PSUM"))

        # per-group state carried across the software pipeline
        groups = [dict() for _ in range(n_groups)]

        # pack whole groups into DMA chunks of <= CHUNK columns
        chunk_groups = []
        cur = []
        cur_sz = 0
        for gi, (g0, gsz) in enumerate(group_bounds):
            if cur and cur_sz + gsz > CHUNK:
                chunk_groups.append(cur)
                cur, cur_sz = [], 0
            cur.append(gi)
            cur_sz += gsz
        if cur:
            chunk_groups.append(cur)

        x_chunks = {}
        h_chunks = {}
        o_chunks = {}
        chunk_of = {}

        def emit_chunk_dma(ci, gis):
            pos = group_bounds[gis[0]][0]
            csz = sum(group_bounds[g][1] for g in gis)
            x_sb = io.tile([H, csz], bf16, tag="x")
            h_sb = io.tile([H, csz], bf16, tag="h")
            nc.sync.dma_start(out=x_sb, in_=xT[:, pos : pos + csz])
            nc.sync.dma_start(out=h_sb, in_=hT[:, pos : pos + csz])
            o_sb = outp.tile([H, csz], bf16, tag="o")
            for gi in gis:
                chunk_of[gi] = (ci, pos, csz)
                x_chunks[gi] = x_sb
                h_chunks[gi] = h_sb
                o_chunks[gi] = o_sb

        # chunk0's x/h go out first so the first matmuls start ASAP, then
        # the small weight/bias loads (HWDGE on the sync engine too), then
        # the remaining chunks stream behind
        emit_chunk_dma(0, chunk_groups[0])
        w_ih_sb = singles.tile([H, 3 * H], bf16)
        nc.sync.dma_start(out=w_ih_sb, in_=w_ihT[:, :])
        w_hh_sb = singles.tile([H, 3 * H], bf16)
        nc.sync.dma_start(out=w_hh_sb, in_=w_hhT[:, :])
        b_sb = singles.tile([H, 4], f32)
        nc.sync.dma_start(out=b_sb, in_=biases[:, :])

        # dummy sigmoid fires the ~2.7us ACT table load immediately, so it
        # overlaps the DMA ramp instead of stalling the first real sigmoid
        warm_sb = singles.tile([H, 1], f32)
        nc.scalar.activation(out=warm_sb, in_=b_sb[:, 0:1],
                             func=sig, bias=0.0, scale=1.0)

        for ci in range(1, len(chunk_groups)):
            emit_chunk_dma(ci, chunk_groups[ci])
        n_chunks = len(chunk_groups)

        W_R, W_Z, W_N = slice(0, H), slice(H, 2 * H), slice(2 * H, 3 * H)

        def emit_gates_mm(g):
            """r/z/hn gate matmuls for group g (10 MMs of FD=512)."""
            st = groups[g]
            x_sb, h_sb = x_chunks[g], h_chunks[g]
            _, cpos, _ = chunk_of[g]
            gstart, gsz = group_bounds[g]
            off = gstart - cpos
            p_r = psum.tile([H, NTILE], f32, tag="p_r")
            p_z = psum.tile([H, NTILE], f32, tag="p_z")
            p_hn = psum_hn.tile([H, NTILE], f32, tag="p_hn")
            st.update(p_r=p_r, p_z=p_z, p_hn=p_hn, off=off, gsz=gsz)
            # weight-major order: all matmuls of one stationary weight run
            # back-to-back, so only one LDWEIGHTS per weight matrix (the
            # LW<->MM toggle otherwise costs ~100ns per matmul on the PE).
            # r-gate first so its sigmoid can start while z / h_n run.
            qslices = [(slice(off + q0, off + q0 + min(512, gsz - q0)),
                        slice(q0, q0 + min(512, gsz - q0)))
                       for q0 in range(0, gsz, 512)]
            for qs, qd in qslices:
                nc.tensor.matmul(p_r[:, qd], w_ih_sb[:, W_R], x_sb[:, qs],
                                 start=True, stop=False)
            for qs, qd in qslices:
                nc.tensor.matmul(p_r[:, qd], w_hh_sb[:, W_R], h_sb[:, qs],
                                 start=False, stop=True)
            for qs, qd in qslices:
                nc.tensor.matmul(p_z[:, qd], w_ih_sb[:, W_Z], x_sb[:, qs],
                                 start=True, stop=False)
            for qs, qd in qslices:
                nc.tensor.matmul(p_z[:, qd], w_hh_sb[:, W_Z], h_sb[:, qs],
                                 start=False, stop=True)
            for qs, qd in qslices:
                nc.tensor.matmul(p_hn[:, qd], w_hh_sb[:, W_N], h_sb[:, qs],
                                 start=True, stop=True)

        def emit_in_mm(g):
            """i_n matmuls accumulating onto (h_n + b)*r already in PSUM."""
            st = groups[g]
            x_sb = x_chunks[g]
            off = st["off"]
            p_hn = st["p_hn"]
            for q0 in range(0, st["gsz"], 512):
                qn = min(512, st["gsz"] - q0)
                nc.tensor.matmul(p_hn[:, q0 : q0 + qn],
                                 w_ih_sb[:, W_N],
                                 x_sb[:, off + q0 : off + q0 + qn],
                                 start=False, stop=True)

        def emit_sig_r(g):
            st = groups[g]
            gsz = st["gsz"]
            r_t = mids.tile([H, NTILE], bf16, tag="r")
            nc.scalar.activation(out=r_t[:, :gsz], in_=st["p_r"][:, :gsz],
                                 func=sig, bias=b_sb[:, 0:1], scale=1.0)
            st["r"] = r_t

        def emit_sig_w(g):
            st = groups[g]
            gsz = st["gsz"]
            w_t = mids.tile([H, NTILE], bf16, tag="w")
            nc.scalar.activation(out=w_t[:, :gsz], in_=st["p_z"][:, :gsz],
                                 func=sig, bias=b_sb[:, 1:2], scale=-1.0)
            st["w"] = w_t

        def emit_stt(g):
            """In-place PSUM rewrite: p_hn <- (p_hn + b_hn) * r."""
            st = groups[g]
            gsz = st["gsz"]
            nc.vector.scalar_tensor_tensor(
                out=st["p_hn"][:, :gsz], in0=st["p_hn"][:, :gsz],
                scalar=b_sb[:, 2:3], in1=st["r"][:, :gsz],
                op0=add_op, op1=mult_op)

        def emit_tanh(g):
            st = groups[g]
            gsz = st["gsz"]
            n_t = mids.tile([H, NTILE], bf16, tag="n")
            nc.scalar.activation(out=n_t[:, :gsz], in_=st["p_hn"][:, :gsz],
                                 func=tanh, bias=b_sb[:, 3:4], scale=1.0)
            st["n"] = n_t

        def emit_blend(g):
            """out = h + w * (n - h): DVE sub, DVE mul, GpSimd add."""
            st = groups[g]
            h_sb, o_sb = h_chunks[g], o_chunks[g]
            off, gsz = st["off"], st["gsz"]
            hs = slice(off, off + gsz)
            m_t = mids.tile([H, NTILE], bf16, tag="m")
            d_t = mids.tile([H, NTILE], bf16, tag="d")
            nc.vector.tensor_sub(out=m_t[:, :gsz], in0=st["n"][:, :gsz],
                                 in1=h_sb[:, hs])
            nc.vector.tensor_mul(out=d_t[:, :gsz], in0=st["w"][:, :gsz],
                                 in1=m_t[:, :gsz])
            nc.gpsimd.tensor_add(out=o_sb[:, hs], in0=d_t[:, :gsz],
                                 in1=h_sb[:, hs])
            # drop references so tile pool can recycle
            st.clear()

        def emit_out_dma(ci_, cpos, csz, o_sb):
            nc.sync.dma_start(out=outT[:, cpos : cpos + csz], in_=o_sb)

        # software pipeline, one group deep on the PE/ACT streams:
        #   g:   gates MM(g), sigmoids(g), stt(g)
        #        [then next iteration emits gates MM(g+1) before in_mm(g)]
        #   g-1 style: in_mm(g), tanh(g), blend(g)
        last_group_of_chunk = {}
        for gi in range(n_groups):
            last_group_of_chunk[chunk_of[gi][0]] = gi

        def maybe_out_dma(g):
            ci_, cpos, csz = chunk_of[g]
            if last_group_of_chunk[ci_] == g:
                emit_out_dma(ci_, cpos, csz, o_chunks[g])

        # Steady-state iteration g (one group deep on PE/ACT/DVE streams):
        #   PE:  in_mm(g-1) first (tanh(g-1) needs it early), then gates(g)
        #   ACT: sig_r(g), tanh(g-1), sig_w(g)  - tanh's STT/in_mm chain had
        #        a full iteration of slack, so ACT runs back-to-back
        #   DVE: stt(g) (after sig_r), then blend muls of g-1 (after tanh)
        #   GpSimd: final add of g-1
        for g in range(n_groups):
            if g > 0:
                emit_in_mm(g - 1)
            emit_gates_mm(g)
            emit_sig_r(g)
            if g > 0:
                emit_tanh(g - 1)
            emit_sig_w(g)
            emit_stt(g)
            if g > 0:
                emit_blend(g - 1)
                maybe_out_dma(g - 1)
        g = n_groups - 1
        emit_in_mm(g)
        emit_tanh(g)
        emit_blend(g)
        maybe_out_dma(g)

    _patch_json(nc)
    return nc


def _get_nc(bpc: int, used: int) -> bass.Bass:
    if (bpc, used) not in _NC_CACHE:
        _NC_CACHE[(bpc, used)] = _build_nc(bpc, used)
    return _NC_CACHE[(bpc, used)]


def kernel(node_ids, messages, memory, W_ih, W_hh, b_ih, b_hh):
    global LAST_RESULT
    node_ids = np.asarray(node_ids)
    messages = np.asarray(messages, dtype=np.float32)
    memory = np.asarray(memory, dtype=np.float32)
    W_ih = np.asarray(W_ih, dtype=np.float32)
    W_hh = np.asarray(W_hh, dtype=np.float32)
    b_ih = np.asarray(b_ih, dtype=np.float32)
    b_hh = np.asarray(b_hh, dtype=np.float32)

    B = node_ids.shape[0]
    per = -(-B // N_CORES)                       # rows per core (unpadded)
    bpc = -(-per // 512) * 512                   # padded to 512 multiple
    nc = _get_nc(bpc, per)

    current = memory[node_ids]                   # [B, H] host gather

    w_ihT = np.ascontiguousarray(W_ih.T).astype(BF16)
    w_hhT = np.ascontiguousarray(W_hh.T).astype(BF16)
    bias = np.empty((H, 4), dtype=np.float32)
    bias[:, 0] = b_ih[0:H] + b_hh[0:H]
    bias[:, 1] = -(b_ih[H : 2 * H] + b_hh[H : 2 * H])
    bias[:, 2] = b_hh[2 * H : 3 * H]
    bias[:, 3] = b_ih[2 * H : 3 * H]

    in_maps = []
    for c in range(N_CORES):
        lo = c * per
        hi = min(lo + per, B)
        xT = np.zeros((H, bpc), dtype=BF16)
        hT = np.zeros((H, bpc), dtype=BF16)
        if hi > lo:
            xT[:, : hi - lo] = messages[lo:hi].T
            hT[:, : hi - lo] = current[lo:hi].T
        in_maps.append({
            "xT": xT, "hT": hT,
            "w_ihT": w_ihT, "w_hhT": w_hhT, "biases": bias,
        })

    res = run_bass_kernel_spmd(nc, in_maps, list(range(N_CORES)))
    LAST_RESULT = res

    updated = np.empty((B, H), dtype=np.float32)
    for c in range(N_CORES):
        lo = c * per
        hi = min(lo + per, B)
        if hi > lo:
            updated[lo:hi] = res.results[c]["outT"][:, : hi - lo].T.astype(np.float32)

    new_memory = memory.copy()
    new_memory[node_ids] = updated
    return new_memory
